# revision 19
# baseline (speedup 1.0000x reference)
"""GNN message-passing (3x GraphConv+BN+ReLU, final GraphConv) on 8 trn2 cores.

Strategy (graph/data parallel, per sharding hint):
  - Nodes are partitioned across 8 cores x 49 chunks x 128 slots, balancing
    in-degree per chunk (LPT) so every chunk has ~E/392 incoming edges.
  - Per layer: AllGather the node features (fp16, node-major) into a
    replicated DRAM table; each core gathers its chunks' edge source rows
    via indirect DMA, reduces them with one-hot matmuls accumulated in PSUM
    (segment-sum), applies the dense transforms in fp32, computes BatchNorm
    stats locally + a tiny AllReduce, then applies BN+ReLU fused on the
    scalar engine.
  - Activations stay transposed ([feat, node]) on-chip; a PE transpose
    rebuilds the node-major fp16 replica for the next layer's gathers.
"""

import os
import sys
import heapq

import numpy as np

sys.path.insert(0, "/opt/trn_rl_repo")

import concourse.bass as bass  # noqa: E402
import concourse.mybir as mybir  # noqa: E402
import concourse.tile as tile  # noqa: E402
from concourse.bass import IndirectOffsetOnAxis  # noqa: E402
from concourse.vector_clock import ScopedClock  # noqa: E402
from concourse import library_config  # noqa: E402
from concourse.library_overlay import lower_extended_insts  # noqa: E402

N = 50000
E = 800000
D = 128
L = 3
OUT = 2
EPS = 1e-5
N_CORES = 8
CHUNKS = 49            # chunks (dst windows of 128 nodes) per core
P = 128
SLOTS_PER_CORE = CHUNKS * P      # 6272 (includes 22 pad slots)
SHORT_SLOTS = N // N_CORES - (CHUNKS - 1) * P   # 106 real nodes in last chunk
N_PAD = N_CORES * SLOTS_PER_CORE  # 50176
SPLIT_LO = 32768          # lo gather table = rows [0, 32768)
HI_BASE = N_PAD - 32768   # hi gather table = rows [17408, 50176)

F16 = mybir.dt.float16
F32 = mybir.dt.float32
I32 = mybir.dt.int32

# ---------------------------------------------------------------------------
# walrus in this container accepts at most ONE semaphore wait per instruction.
# Patch the Tile exit drain and add a post-pass splitting multi-wait insts.
# ---------------------------------------------------------------------------
_MAX_WAITS = 1


def _drain_and_barrier(self, tick_clock, wait_clock):
    nc = self.nc
    drain_inst = nc.sync.drain()
    wait_clock.add_sem_waits(
        drain_inst.ins, ScopedClock({None: tick_clock.global_clock})
    )
    si = drain_inst.ins.sync_info
    if si is not None and si.on_wait is not None and len(si.on_wait) > _MAX_WAITS:
        waits = list(si.on_wait)
        si.on_wait = waits[:_MAX_WAITS]
        rest = waits[_MAX_WAITS:]
        for i in range(0, len(rest), _MAX_WAITS):
            nop = nc.sync.nop(nofuse=True)
            nop.ins.sync_info = mybir.SyncInfo(
                on_wait=rest[i : i + _MAX_WAITS], on_update=[]
            )
    nc.all_engine_barrier()
    assert self.sems is not None
    popped = nc._tile_sem_poison_stack.pop()
    assert popped is self._sem_poison
    nc.clear_and_free_semaphores(list(self.sems.allocated().values()))
    nc.all_engine_barrier()


tile.TileContext._drain_and_barrier = _drain_and_barrier


def _split_multiwait(nc):
    n_split = 0
    for fn in nc.m.functions:
        for blk in fn.blocks:
            out = []
            for inst in blk.instructions:
                si = inst.sync_info
                if si is not None and si.on_wait and len(si.on_wait) > _MAX_WAITS:
                    waits = list(si.on_wait)
                    si.on_wait = waits[-_MAX_WAITS:]
                    rest = waits[:-_MAX_WAITS]
                    for i in range(0, len(rest), _MAX_WAITS):
                        n_split += 1
                        out.append(
                            mybir.InstNoOp(
                                name=f"{inst.name}-ws{i}",
                                engine=inst.engine,
                                ins=[],
                                outs=[],
                                bass_nofuse=True,
                                sync_info=mybir.SyncInfo(
                                    on_wait=rest[i : i + _MAX_WAITS], on_update=[]
                                ),
                                debug=inst.debug,
                            )
                        )
                out.append(inst)
            blk.instructions[:] = out
    return n_split


# ---------------------------------------------------------------------------
# Host-side graph partitioning
# ---------------------------------------------------------------------------
def _partition_nodes(deg):
    """Assign each node to (bin, slot); bins are (core, chunk) with 128 slots
    (106 for each core's last chunk), LPT-balancing in-degree per bin."""
    n_bins = N_CORES * CHUNKS
    caps = np.full(n_bins, P, np.int64)
    caps[CHUNKS - 1 :: CHUNKS] = SHORT_SLOTS
    order = np.argsort(-deg, kind="stable")
    bin_of = np.empty(N, np.int32)
    slot_of = np.empty(N, np.int32)
    fill = np.zeros(n_bins, np.int64)
    sums = np.zeros(n_bins, np.int64)
    heap = [(0, b) for b in range(n_bins)]
    heapq.heapify(heap)
    for node in order:
        d = int(deg[node])
        while True:
            s, b = heapq.heappop(heap)
            if fill[b] < caps[b]:
                break
        bin_of[node] = b
        slot_of[node] = fill[b]
        fill[b] += 1
        sums[b] += d
        if fill[b] < caps[b]:
            heapq.heappush(heap, (int(sums[b]), b))
    return bin_of, slot_of, int(sums.max())


def _preprocess(x, edge_index):
    x = np.asarray(x, np.float32)
    ei = np.asarray(edge_index)
    src = ei[0].astype(np.int64)
    dst = ei[1].astype(np.int64)
    deg = np.bincount(dst, minlength=N)
    bin_of, slot_of, max_bin_edges = _partition_nodes(deg)
    KT = max(16, -(-max_bin_edges // P))  # tiles of 128 edges per chunk

    core_of = bin_of // CHUNKS
    chunk_of = bin_of % CHUNKS
    newid = core_of.astype(np.int64) * SLOTS_PER_CORE + (
        chunk_of.astype(np.int64) * P + slot_of
    )

    KH = (KT + 1) // 2          # gather tiles per table half
    KT = 2 * KH
    half_cap = KH * P

    # order edges by destination bin; within a bin split across the two
    # dma_gather table halves (int16 index limit), balancing counts via the
    # overlap region [HI_BASE, SPLIT_LO)
    key = bin_of[dst]
    order = np.argsort(key, kind="stable")
    e_bin = key[order]
    e_src_new = newid[src[order]]
    e_dst_slot = slot_of[dst[order]]

    n_bins = N_CORES * CHUNKS
    # per-edge half preference: 0 = lo-only, 2 = hi-only, 1 = overlap
    pref = np.where(e_src_new < HI_BASE, 0, np.where(e_src_new >= SPLIT_LO, 2, 1))
    # sort edges by (bin, pref) so lo-only first, overlap middle, hi-only last
    order2 = np.lexsort((pref, e_bin))
    e_bin = e_bin[order2]
    e_src_new = e_src_new[order2]
    e_dst_slot = e_dst_slot[order2]
    pref = pref[order2]

    counts = np.bincount(e_bin, minlength=n_bins)
    starts = np.concatenate([[0], np.cumsum(counts)[:-1]])
    lo_only = np.bincount(e_bin[pref == 0], minlength=n_bins)
    hi_only = np.bincount(e_bin[pref == 2], minlength=n_bins)
    # lo half gets n_lo edges: the first n_lo of the bin's (sorted) edges
    # grow KH until the per-half split is feasible for every bin
    while True:
        half_cap = KH * P
        lo_min = np.maximum(lo_only, counts - half_cap)
        lo_max = np.minimum(counts - hi_only, half_cap)
        if (lo_min <= lo_max).all():
            break
        KH += 1
    KT = 2 * KH
    n_lo = np.clip((counts + 1) // 2, lo_min, lo_max)

    pos_in_bin = np.arange(E) - starts[e_bin]
    in_lo = pos_in_bin < n_lo[e_bin]
    # slot within the bin's 2*half_cap edge layout: lo edges at [0, half_cap),
    # hi edges at [half_cap, 2*half_cap)
    slot = np.where(in_lo, pos_in_bin, half_cap + (pos_in_bin - n_lo[e_bin]))

    cap = 2 * half_cap
    idx_big = np.zeros(n_bins * cap, np.int16)
    dst_big = np.full(n_bins * cap, -1.0, np.float16)
    flat = e_bin.astype(np.int64) * cap + slot
    rel = np.where(in_lo, e_src_new, e_src_new - HI_BASE)
    assert rel.max() < SPLIT_LO and rel.min() >= 0
    idx_big[flat] = rel.astype(np.int16)
    dst_big[flat] = e_dst_slot.astype(np.float16)

    # dst_loc: [n_bins, KT, 128] -> per core [128, CHUNKS*KT] with [p, c*KT+t]
    dst_r = dst_big.reshape(N_CORES, CHUNKS, KT, P).transpose(0, 3, 1, 2)
    dst_cores = np.ascontiguousarray(dst_r.reshape(N_CORES, P, CHUNKS * KT))

    # idx arrays for dma_gather: logical index i lives at idxs[i % 16, i // 16],
    # rows replicated to 128 partitions. Chunks are gathered in groups of
    # GRP=7: per group the 7 chunks' lo column-blocks come first, then the
    # 7 hi blocks (matching the two grouped dma_gather calls).
    GRP = 7
    assert CHUNKS == GRP * GRP
    cols = half_cap // 16
    # [n_bins, half, seq] -> wrapped [n_bins, half, 16, cols]
    idx_w = idx_big.reshape(n_bins, 2, cols, 16).transpose(0, 1, 3, 2)
    idx_g = idx_w.reshape(N_CORES, GRP, GRP, 2, 16, cols).transpose(0, 1, 3, 2, 4, 5)
    # -> [core, group, half, chunk_in_group, 16, cols]; replicate rows to 128
    idx_g = np.broadcast_to(
        idx_g[:, :, :, :, None, :, :],
        (N_CORES, GRP, 2, GRP, 8, 16, cols),
    ).reshape(N_CORES, GRP, 2, GRP, P, cols)
    idx_cores = np.ascontiguousarray(
        idx_g.transpose(0, 4, 1, 2, 3, 5).reshape(N_CORES, P, CHUNKS * 2 * cols)
    )

    # permuted node features (replicated fp16 table + per-core transposed fp32)
    x_pad = np.zeros((N_PAD, D), np.float32)
    x_pad[newid] = x
    x_rep = np.ascontiguousarray(x_pad.astype(np.float16))
    xT_loc = np.ascontiguousarray(
        x_pad.reshape(N_CORES, SLOTS_PER_CORE, D).transpose(0, 2, 1).astype(np.float32)
    )
    return KT, newid, idx_cores, dst_cores, x_rep, xT_loc


# ---------------------------------------------------------------------------
# Device program
# ---------------------------------------------------------------------------
def build_program(KT):
    nc = bass.Bass(num_devices=N_CORES)
    S_COLS = CHUNKS * KT

    p_xrep = nc.declare_dram_parameter("x_rep", [N_PAD, D], F16, isOutput=False)
    p_xT = nc.declare_dram_parameter("xT_loc", [D, SLOTS_PER_CORE], F32, isOutput=False)
    KH = KT // 2
    I_COLS = CHUNKS * 2 * (KH * P // 16)
    p_idx = nc.declare_dram_parameter("gidx", [P, I_COLS], mybir.dt.int16, isOutput=False)
    p_dst = nc.declare_dram_parameter("dst_loc", [P, S_COLS], F16, isOutput=False)
    p_wrel = nc.declare_dram_parameter("wrel", [L, D, D], F32, isOutput=False)
    p_wroot = nc.declare_dram_parameter("wroot", [L, D, D], F32, isOutput=False)
    p_wrel2 = nc.declare_dram_parameter("wrel2", [D, OUT], F32, isOutput=False)
    p_wroot2 = nc.declare_dram_parameter("wroot2", [D, OUT], F32, isOutput=False)
    p_bR = nc.declare_dram_parameter("bR", [1, L * D], F32, isOutput=False)
    p_gammaT = nc.declare_dram_parameter("gammaT", [D, L], F32, isOutput=False)
    p_betaT = nc.declare_dram_parameter("betaT", [D, L], F32, isOutput=False)
    p_b2 = nc.declare_dram_parameter("b2", [1, OUT], F32, isOutput=False)
    p_iota = nc.declare_dram_parameter("iota16", [P, P], F16, isOutput=False)
    p_ident = nc.declare_dram_parameter("ident32", [P, P], F32, isOutput=False)
    p_out = nc.declare_dram_parameter("z4T", [OUT, SLOTS_PER_CORE], F32, isOutput=True)

    rg = [list(range(N_CORES))]

    with tile.TileContext(nc) as tc:
        with (
            tc.tile_pool(name="dram_rep", bufs=2, space="DRAM") as dram_rep,
            tc.tile_pool(name="dram_ag", bufs=2, space="DRAM") as dram_ag,
            tc.tile_pool(name="dram_cc", bufs=2, space="DRAM") as dram_cc,
            tc.tile_pool(name="singles", bufs=1) as singles,
            tc.tile_pool(name="hT", bufs=2) as hT_pool,
            tc.tile_pool(name="zb", bufs=1) as z_pool,
            tc.tile_pool(name="gath", bufs=2) as g_pool,
            tc.tile_pool(name="sel", bufs=3) as s_pool,
            tc.tile_pool(name="agg", bufs=2) as agg_pool,
            tc.tile_pool(name="t16p", bufs=2) as t16_pool,
            tc.tile_pool(name="bns", bufs=2) as bn_pool,
            tc.tile_pool(name="stat", bufs=2) as stat_pool,
            tc.tile_pool(name="psA", bufs=2, space="PSUM") as psA,
            tc.tile_pool(name="psZ", bufs=2, space="PSUM") as psZ,
            tc.tile_pool(name="psT", bufs=2, space="PSUM") as psT,
        ):
            with tc.high_priority():
                nc.gpsimd.load_library(library_config.mlp)
            nidx_regs = {}
            for ng in sorted({min(7, CHUNKS - c) for c in range(0, CHUNKS, 7)}):
                nidx_regs[ng] = nc.gpsimd.to_reg(ng * KH * P)
            # --- constants / weights in SBUF ---
            idx_sb = singles.tile([P, I_COLS], mybir.dt.int16)
            nc.sync.dma_start(out=idx_sb[:], in_=p_idx[:])
            dst_sb = singles.tile([P, S_COLS], F16)
            nc.sync.dma_start(out=dst_sb[:], in_=p_dst[:])
            iota_sb = singles.tile([P, P], F16)
            nc.sync.dma_start(out=iota_sb[:], in_=p_iota[:])
            ident_sb = singles.tile([P, P], F32)
            nc.sync.dma_start(out=ident_sb[:], in_=p_ident[:])
            wrel_sb = singles.tile([P, L * D], F32)
            wroot_sb = singles.tile([P, L * D], F32)
            for l in range(L):
                nc.sync.dma_start(out=wrel_sb[:, l * D : (l + 1) * D], in_=p_wrel[l])
                nc.sync.dma_start(out=wroot_sb[:, l * D : (l + 1) * D], in_=p_wroot[l])
            w2_sb = singles.tile([P, 2 * OUT], F32)
            nc.sync.dma_start(out=w2_sb[:, :OUT], in_=p_wrel2[:])
            nc.sync.dma_start(out=w2_sb[:, OUT : 2 * OUT], in_=p_wroot2[:])
            bR_sb = singles.tile([1, L * D], F32)
            nc.sync.dma_start(out=bR_sb[:], in_=p_bR[:])
            ones_sb = singles.tile([1, P], F32)
            nc.vector.memset(ones_sb[:], 1.0)
            gammaT_sb = singles.tile([P, L], F32)
            nc.sync.dma_start(out=gammaT_sb[:], in_=p_gammaT[:])
            betaT_sb = singles.tile([P, L], F32)
            nc.sync.dma_start(out=betaT_sb[:], in_=p_betaT[:])
            b2_sb = singles.tile([1, OUT], F32)
            nc.sync.dma_start(out=b2_sb[:], in_=p_b2[:])
            eps_sb = singles.tile([P, 1], F32)
            nc.vector.memset(eps_sb[:], EPS)

            # initial hT (fp32, [feat, slot]) and h replica (fp16 node-major)
            hT_prev = hT_pool.tile([P, SLOTS_PER_CORE], F32, tag="hT")
            nc.sync.dma_start(out=hT_prev[:], in_=p_xT[:])

            h_rep = p_xrep

            out_sb = singles.tile([OUT, SLOTS_PER_CORE], F32)

            for l in range(L + 1):
                is_final = l == L
                if is_final:
                    w_rel = w2_sb[:, :OUT]
                    w_root = w2_sb[:, OUT : 2 * OUT]
                else:
                    w_rel = wrel_sb[:, l * D : (l + 1) * D]
                    w_root = wroot_sb[:, l * D : (l + 1) * D]

                z_all = None if is_final else z_pool.tile([P, SLOTS_PER_CORE], F32)
                stats = None if is_final else stat_pool.tile(
                    [P, CHUNKS, nc.vector.BN_STATS_DIM], F32
                )

                icols = KH * P // 16   # idx columns per half
                GRP = 7                # chunks gathered per dma_gather pair
                gath_group = None
                for c in range(CHUNKS):
                    gi = c % GRP
                    if gi == 0:
                        ng = min(GRP, CHUNKS - c)
                        gath_group = g_pool.tile([P, GRP * KT * P], F16)
                        gg = gath_group.rearrange(
                            "p (g t d) -> p (g t) d", g=GRP, t=KT
                        )
                        # lo halves of ng chunks in one call: idx columns for
                        # chunks c..c+ng-1 lo are interleaved (2c, 2c+2, ...)
                        # -> host packs them contiguously; see _preprocess.
                        nc.gpsimd.dma_gather(
                            out_ap=gg[:, 0 : ng * KH, :],
                            in_ap=h_rep[0:SPLIT_LO, :],
                            idxs_ap=idx_sb[
                                :, (2 * c) * icols : (2 * c + ng) * icols
                            ],
                            num_idxs=ng * KH * P,
                            num_idxs_reg=nidx_regs[ng],
                            elem_size=D,
                            single_packet=False,
                        )
                        nc.gpsimd.dma_gather(
                            out_ap=gg[:, ng * KH : 2 * ng * KH, :],
                            in_ap=h_rep[HI_BASE:N_PAD, :],
                            idxs_ap=idx_sb[
                                :, (2 * c + ng) * icols : (2 * c + 2 * ng) * icols
                            ],
                            num_idxs=ng * KH * P,
                            num_idxs_reg=nidx_regs[ng],
                            elem_size=D,
                            single_packet=False,
                        )
                        ng_cur = ng
                    # chunk c's lo tiles at group slots [gi*KH, (gi+1)*KH),
                    # hi tiles at [ng*KH + gi*KH, ...)
                    gath = None
                    lo0 = gi * KH * P
                    hi0 = (ng_cur + gi) * KH * P
                    sel = s_pool.tile([P, KT * P], F16)
                    dst_bc = bass.AP(
                        tensor=dst_sb.tensor,
                        offset=dst_sb[:, c * KT : (c + 1) * KT].offset,
                        ap=list(dst_sb[:, c * KT : (c + 1) * KT].ap) + [[0, P]],
                    )
                    iota_bc = bass.AP(
                        tensor=iota_sb.tensor,
                        offset=iota_sb[:].offset,
                        ap=[iota_sb[:].ap[0], [0, KT], iota_sb[:].ap[1]],
                    )
                    nc.vector.tensor_tensor(
                        out=sel.rearrange("p (t w) -> p t w", t=KT),
                        in0=dst_bc,
                        in1=iota_bc,
                        op=mybir.AluOpType.is_equal,
                    )
                    ps_a = psA.tile([P, P], F32, space="PSUM")
                    for t in range(KT):
                        if t < KH:
                            gsl = slice(lo0 + t * P, lo0 + (t + 1) * P)
                        else:
                            gsl = slice(
                                hi0 + (t - KH) * P, hi0 + (t - KH + 1) * P
                            )
                        nc.tensor.matmul(
                            out=ps_a[:],
                            lhsT=gath_group[:, gsl],
                            rhs=sel[:, t * P : (t + 1) * P],
                            start=(t == 0),
                            stop=(t == KT - 1),
                        )
                    aggT = agg_pool.tile([P, P], F32)
                    nc.scalar.activation(
                        out=aggT[:], in_=ps_a[:],
                        func=mybir.ActivationFunctionType.Copy,
                    )

                    cs = slice(c * P, (c + 1) * P)
                    if is_final:
                        ps_z = psZ.tile([OUT, P], F32, space="PSUM")
                    else:
                        ps_z = psZ.tile([P, P], F32, space="PSUM")
                    nc.tensor.matmul(
                        out=ps_z[:], lhsT=w_rel, rhs=aggT[:], start=True, stop=False
                    )
                    nc.tensor.matmul(
                        out=ps_z[:], lhsT=w_root, rhs=hT_prev[:, cs],
                        start=False, stop=False,
                    )
                    nc.tensor.matmul(
                        out=ps_z[:],
                        lhsT=b2_sb[:] if is_final else bR_sb[:, l * D : (l + 1) * D],
                        rhs=ones_sb[:],
                        start=False, stop=True,
                    )
                    if is_final:
                        nc.scalar.activation(
                            out=out_sb[:, cs], in_=ps_z[:],
                            func=mybir.ActivationFunctionType.Copy,
                        )
                    else:
                        nc.scalar.activation(
                            out=z_all[:, cs], in_=ps_z[:],
                            func=mybir.ActivationFunctionType.Copy,
                        )
                        width = SHORT_SLOTS if c == CHUNKS - 1 else P
                        nc.vector.bn_stats(
                            out=stats[:, c, :],
                            in_=z_all[:, c * P : c * P + width],
                        )

                if is_final:
                    nc.sync.dma_start(out=p_out[:], in_=out_sb[:])
                    continue

                # ---- BatchNorm over all nodes ----
                bs = bn_pool.tile([P, 16], F32)
                mv = bs[:, 0:2]
                nc.vector.bn_aggr(out=mv, in_=stats[:])
                # send [mean_c, E2_c]; per-core counts are equal so the
                # global stats are plain means of the 8 entries
                cc_sb = bs[:, 3:5]
                nc.vector.tensor_copy(out=cc_sb[:, 0:1], in_=mv[:, 0:1])
                nc.vector.tensor_scalar(
                    out=cc_sb[:, 1:2], in0=mv[:, 0:1], scalar1=mv[:, 0:1],
                    scalar2=mv[:, 1:2], op0=mybir.AluOpType.mult,
                    op1=mybir.AluOpType.add,
                )
                cc_in = dram_cc.tile([P, 2], F32)
                cc_out = dram_cc.tile([P * N_CORES, 2], F32, addr_space="Shared")
                nc.sync.dma_start(out=cc_in[:], in_=cc_sb)
                nc.gpsimd.collective_compute(
                    "AllGather", mybir.AluOpType.bypass, replica_groups=rg,
                    ins=[cc_in.opt()], outs=[cc_out.opt()],
                )
                # cc_out rows = rank*128 + p; load as [p, (j r)] and reduce r
                cc_all = bn_pool.tile([P, 2, N_CORES], F32)
                cc_src = bass.AP(
                    tensor=cc_out.tensor,
                    offset=cc_out[:].offset,
                    ap=[[2, P], [1, 2], [2 * P, N_CORES]],
                )
                nc.sync.dma_start(out=cc_all[:], in_=cc_src)
                cc_res = bs[:, 5:7]
                nc.vector.tensor_reduce(
                    out=cc_res.rearrange("p (a b) -> p a b", a=2),
                    in_=cc_all[:],
                    axis=mybir.AxisListType.X,
                    op=mybir.AluOpType.add,
                )

                mu = bs[:, 7:8]
                nc.vector.tensor_scalar(
                    out=mu, in0=cc_res[:, 0:1], scalar2=None,
                    op0=mybir.AluOpType.mult, scalar1=1.0 / N_CORES,
                )
                var = bs[:, 8:9]
                nc.vector.tensor_scalar(
                    out=var, in0=cc_res[:, 1:2], scalar2=None,
                    op0=mybir.AluOpType.mult, scalar1=1.0 / N_CORES,
                )
                mu2 = bs[:, 9:10]
                nc.vector.tensor_tensor(
                    out=mu2, in0=mu, in1=mu, op=mybir.AluOpType.mult
                )
                nc.vector.tensor_tensor(
                    out=var, in0=var, in1=mu2, op=mybir.AluOpType.subtract
                )
                rstd = bs[:, 10:11]
                nc.scalar.activation(
                    out=rstd, in_=var,
                    func=mybir.ActivationFunctionType.Sqrt,
                    bias=eps_sb[:], scale=1.0,
                )
                nc.vector.reciprocal(out=rstd, in_=rstd)
                scale = bs[:, 11:12]
                nc.vector.tensor_tensor(
                    out=scale, in0=rstd, in1=gammaT_sb[:, l : l + 1],
                    op=mybir.AluOpType.mult,
                )
                shift = bs[:, 12:13]
                nc.vector.tensor_tensor(
                    out=shift, in0=mu, in1=scale, op=mybir.AluOpType.mult
                )
                nc.vector.tensor_tensor(
                    out=shift, in0=betaT_sb[:, l : l + 1], in1=shift,
                    op=mybir.AluOpType.subtract,
                )

                # BN apply + relu (one op), then transposes staged 7 chunks
                # per ag_in DMA to cut HWDGE call count
                hT_new = hT_pool.tile([P, SLOTS_PER_CORE], F32, tag="hT")
                ag_in = dram_ag.tile([SLOTS_PER_CORE, D], F16)
                t16g = None
                for c in range(CHUNKS):
                    gi = c % 7
                    if gi == 0:
                        t16g = t16_pool.tile([P, 7, P], F16)
                        gs = slice(c * P, (c + 7) * P)
                        nc.scalar.activation(
                            out=hT_new[:, gs], in_=z_all[:, gs],
                            func=mybir.ActivationFunctionType.Relu,
                            bias=shift, scale=scale,
                        )
                    cs2 = slice(c * P, (c + 1) * P)
                    ps_t = psT.tile([P, P], F32, space="PSUM")
                    nc.tensor.transpose(
                        out=ps_t[:], in_=hT_new[:, cs2],
                        identity=ident_sb[:],
                    )
                    nc.vector.tensor_copy(out=t16g[:, gi, :], in_=ps_t[:])
                    if gi == 6:
                        g0 = (c - 6) * P
                        dst = bass.AP(
                            tensor=ag_in.tensor,
                            offset=ag_in[:].offset + g0 * D,
                            ap=[[D, P], [P * D, 7], [1, D]],
                        )
                        nc.sync.dma_start(out=dst, in_=t16g[:])
                h_rep = dram_rep.tile([N_PAD, D], F16, addr_space="Shared")
                nc.gpsimd.collective_compute(
                    "AllGather", mybir.AluOpType.bypass, replica_groups=rg,
                    ins=[ag_in.opt()], outs=[h_rep.opt()],
                )
                hT_prev = hT_new

    lower_extended_insts(nc)
    _split_multiwait(nc)
    return nc


_PROGRAM_CACHE = {}


def _get_program(KT):
    if KT not in _PROGRAM_CACHE:
        _PROGRAM_CACHE[KT] = build_program(KT)
    return _PROGRAM_CACHE[KT]


def _make_in_maps(KT, idx_cores, dst_cores, x_rep, xT_loc,
                  Wrel, Wroot, b, gamma, beta, Wrel2, Wroot2, b2):
    iota16 = np.broadcast_to(np.arange(P, dtype=np.float16), (P, P)).copy()
    ident32 = np.eye(P, dtype=np.float32)
    common = dict(
        wrel=np.ascontiguousarray(np.asarray(Wrel, np.float32)),
        wroot=np.ascontiguousarray(np.asarray(Wroot, np.float32)),
        wrel2=np.ascontiguousarray(np.asarray(Wrel2, np.float32)),
        wroot2=np.ascontiguousarray(np.asarray(Wroot2, np.float32)),
        bR=np.ascontiguousarray(np.asarray(b, np.float32).reshape(1, L * D)),
        gammaT=np.ascontiguousarray(np.asarray(gamma, np.float32).T),
        betaT=np.ascontiguousarray(np.asarray(beta, np.float32).T),
        b2=np.asarray(b2, np.float32).reshape(1, OUT),
        iota16=iota16,
        ident32=ident32,
    )
    in_maps = []
    for c in range(N_CORES):
        m = dict(common)
        m["x_rep"] = x_rep
        m["xT_loc"] = xT_loc[c]
        m["gidx"] = idx_cores[c]
        m["dst_loc"] = dst_cores[c]
        in_maps.append(m)
    return in_maps


def run(x, edge_index, Wrel, Wroot, b, gamma, beta, Wrel2, Wroot2, b2):
    """Returns (output [N, OUT] float32, nc, KT) — nc/KT exposed for profiling."""
    KT, newid, idx_cores, dst_cores, x_rep, xT_loc = _preprocess(x, edge_index)
    nc = _get_program(KT)
    in_maps = _make_in_maps(
        KT, idx_cores, dst_cores, x_rep, xT_loc,
        Wrel, Wroot, b, gamma, beta, Wrel2, Wroot2, b2,
    )
    from concourse.bass_utils import run_bass_kernel_spmd

    res = run_bass_kernel_spmd(nc, in_maps, list(range(N_CORES)))
    full = np.concatenate(
        [res.results[c]["z4T"].T for c in range(N_CORES)], axis=0
    )  # [N_PAD, OUT]
    return full[newid].astype(np.float32), nc, KT


def kernel(**inputs):
    out, _, _ = run(**{k: np.asarray(v) for k, v in inputs.items()})
    return out
